# revision 19
# baseline (speedup 1.0000x reference)
"""Trainium2 Bass kernel for nn_Capsule_16484084482446.

Reference math collapses: with cw = softmax(rw, axis=1),
  outputs[b,j,d] = sum_i sum_n cw[b,i,n] * u[b,j,n,d]
                 = sum_n u[b,j,n,d]           (since sum_i cw[b,i,n] == 1)
so the routing loop is a no-op and the final result is
  out = (sum_n x[b,n,:]) @ W   reshaped to (B, 10, 16).

Kernel strategy (data-parallel over batch, 4 batches per core x 8 cores):
  per core: x_shard (4, 4096, 128) viewed as 128 partitions x (128 rows x 128 d);
  partition p holds rows [128p, 128p+128), so batch b owns partitions [32b, 32b+32).

v2 final (baseline fp32 at 39.9us -> ~36.2us min; measured-trace findings):
  1. x chunks stream on the single sync HWDGE queue IN ORDER: the 16 DMA
     engines saturate (~340-376 GB/s run-to-run) either way, and a single
     queue makes chunk c complete at its stream-position time - splitting
     chunks across both HWDGE queues halved each queue's rate and DOUBLED
     every chunk's completion latency (v2b regression, +2.7us).
  2. DVE fold level 1 writes fp16 (fp32+fp32->fp16 add, measured 690ns
     per 512-elem level); levels 2+ run at the 16-bit 2-elem/cycle rate
     (415/283/215ns vs fp32 684/418/284). Fold work ~21us -> ~14us,
     under the ~22us stream window, so folds track the stream with no
     end-of-stream backlog (baseline lost ~3us here).
     gpsimd folding does NOT help: measured 2.5x slower than DVE and its
     SBUF port pressure slowed concurrent DVE ops up to 4x (v2b).
  3. All matmuls fp16 single-pass: mask-matmul LDWEIGHTS+MATMUL measured
     105+164ns vs fp32 dual-pass 625+700ns; final W-matmul 80+292ns.
     PSUM accumulates fp32.
  4. Chunk schedule minimizes max_c(land_c + sem_prop + suffix folds):
     smooth 16->8->4->2 taper; the last fold is one 284ns op, so the DVE
     finishes ~0.8us after the last byte lands.
  5. W is the LAST DMA on the sync queue (lands ~31.8us); the DVE casts
     it to fp16 in its idle slot between its last fold and the PE's
     final mask-matmul. W anywhere earlier delays the x stream (scalar-
     queue-first: +0.45us; gpsimd SWDGE cast-DMA at head: +4us). Beware:
     a first activation op on the Scalar engine loads the ACT table
     (~1.5us) - don't put the cast there.
  6. No wait on the out-DMA completion semaphore: the block-exit DRAIN
     (measured 160ns) flushes the queue, overlapping the exit barrier
     with the out-DMA flight (saves the ~1.2us land+sem wait).
  Post-fold chain (last byte -> out-DMA issued) is 3.3us; preamble 6.8us
  and exit accounting ~1.5us are runtime-fixed. Run-to-run HBM bandwidth
  varies +-5% (shared chip), so exec time spreads ~36.2-39.8us.
  Accuracy: fp16 folds of ~N(0,1) data with fp32 PSUM accumulation ->
  rel err ~6e-4 measured (gate 2e-2; inputs are seeded the same locally
  and in the harness, so a local pass is deterministic).

Raw Bass (no TileContext): Tile's tail drain needs more sync-wait slots than the
TRN2 CTRL encoding allows for this DMA-lane mix, and its end-of-kernel barriers
would dominate a ~36 us kernel. Every semaphore is cleared by its final consumer
right after its last wait, so the NEFF re-executes cleanly (profilers loop it).
"""

from contextlib import ExitStack

import numpy as np

import concourse.bass as bass
from concourse import mybir
from concourse.bass_utils import run_bass_kernel_spmd

N_CORES = 8
B, N, DIN = 32, 4096, 128
BSH = B // N_CORES          # 4 batches per core
DOUT = 160                  # 10 capsules * 16 dims
# rows-per-partition split: smooth 16->8->4->2 taper. Chosen to minimize
# max_c(land_c + sem_prop + sum of fold times from c to end): every chunk's
# fold finishes right as the next lands, and the last fold is one 284ns op,
# so the DVE is done 0.77us after the last byte (the schedule's floor).
CHUNKS = [8, 8, 16, 16, 16, 16, 8, 8, 8, 4, 4, 4, 4, 2, 2, 2, 2]
assert sum(CHUNKS) == BSH * N // 128
NCHUNK = len(CHUNKS)

F32 = mybir.dt.float32
F16 = mybir.dt.float16

_cache = {}


def _fold(eng, xc_c, xh_c, rows):
    """Halving fold of xc_c (fp32, rows*DIN) into xh_c[:, :DIN] (fp16).
    Level 1 casts fp32->fp16; later levels run at the 2x 16-bit rate."""
    s = rows // 2
    op = eng.tensor_add(
        xh_c[:, : s * DIN], xc_c[:, : s * DIN], xc_c[:, s * DIN :]
    )
    while s > 1:
        s //= 2
        op = eng.tensor_add(
            xh_c[:, : s * DIN],
            xh_c[:, : s * DIN],
            xh_c[:, s * DIN : 2 * s * DIN],
        )
    return op


def _build_nc(chunks=None, out_wait=False):
    global CHUNKS, NCHUNK
    if chunks is not None:
        CHUNKS = chunks
        NCHUNK = len(CHUNKS)
    assert sum(CHUNKS) == BSH * N // 128
    nc = bass.Bass()
    x = nc.dram_tensor("x", [BSH, N, DIN], F32, kind="ExternalInput")
    w = nc.dram_tensor("W", [DIN, DOUT], F32, kind="ExternalInput")
    out = nc.dram_tensor("out", [BSH, DOUT], F32, kind="ExternalOutput")

    # (128, 128, 128): partition p, row-in-partition n, feature d
    x3 = x[:].flatten_outer_dims().rearrange("(p n) d -> p n d", p=128)
    starts = np.cumsum([0] + CHUNKS).tolist()

    with ExitStack() as ctx:
        ec = ctx.enter_context
        xc = [ec(nc.sbuf_tensor(f"xc{c}", [128, CHUNKS[c] * DIN], F32))
              for c in range(NCHUNK)]
        xh = [ec(nc.sbuf_tensor(f"xh{c}", [128, (CHUNKS[c] // 2) * DIN], F16))
              for c in range(NCHUNK)]
        w_sb = ec(nc.sbuf_tensor("w_sb", [DIN, DOUT], F32))
        w16 = ec(nc.sbuf_tensor("w16", [DIN, DOUT], F16))
        mask_sb = ec(nc.sbuf_tensor("mask_sb", [128, BSH], F16))
        s16 = ec(nc.sbuf_tensor("s16", [DIN, BSH], F16))
        out_sb = ec(nc.sbuf_tensor("out_sb", [BSH, DOUT], F32))
        psum_s = ec(nc.psum_tensor("psum_s", [DIN, BSH], F32))
        psum_o = ec(nc.psum_tensor("psum_o", [BSH, DOUT], F32))

        dma_w = ec(nc.semaphore("dma_w"))
        dma_c = [ec(nc.semaphore(f"dma_c{c}")) for c in range(NCHUNK)]
        v_red = ec(nc.semaphore("v_red"))    # +1 per finished DVE fold
        v_w16 = ec(nc.semaphore("v_w16"))    # w16 landed (DMA sem, +16)
        pe_sem = ec(nc.semaphore("pe_sem"))
        v_sem = ec(nc.semaphore("v_sem"))    # s16 ready
        v_out = ec(nc.semaphore("v_out"))
        dma_out = ec(nc.semaphore("dma_out"))  # never waited (drain flushes)
        # Sem hygiene without an entry barrier: every semaphore is cleared by
        # its final consumer right after the consumer's last wait on it, so
        # every run (the profiler re-executes the NEFF) starts from zeros.
        # dma_out only ever grows; nothing waits on an absolute value.
        # no_gpsimd_drain: gpsimd issues no DMAs, so skip its expensive
        # dge_drain at block exit and use the sem-only barrier; Sync still
        # drains, which is what guarantees the out-DMA flushed
        block = ec(nc.Block(no_gpsimd_drain=True))

        @block.sync
        def _(sync):
            # W last: anywhere earlier delays the x stream (W-first on the
            # scalar HWDGE queue cost +0.45us; a gpsimd SWDGE cast-DMA at the
            # head cost +4us of stream delay). It lands ~31.8us, right when
            # the DVE goes idle to cast it.
            for c in range(NCHUNK):
                sync.dma_start(
                    xc[c][:], x3[:, starts[c] : starts[c + 1], :]
                ).then_inc(dma_c[c], 16)
            sync.dma_start(w_sb[:], w[:]).then_inc(dma_w, 16)
            sync.wait_ge(v_out, 1)
            sync.sem_clear(v_out)
            sync.dma_start(out[:], out_sb[:]).then_inc(dma_out, 16)
            if out_wait:
                sync.wait_ge(dma_out, 16)
                sync.sem_clear(dma_out)

        @block.vector
        def _(vector):
            # 0/1 batch mask, one 32-partition quadrant at a time (nonzero
            # partition bases only allow 32-partition windows)
            for q in range(4):
                for b in range(BSH):
                    vector.memset(
                        mask_sb[32 * q : 32 * (q + 1), b : b + 1],
                        1.0 if q == b else 0.0,
                    )
            for c in range(NCHUNK):
                vector.wait_ge(dma_c[c], 16)
                vector.sem_clear(dma_c[c])
                _fold(vector, xc[c], xh[c], CHUNKS[c]).then_inc(v_red, 1)
            # w16 cast sits in the DVE's idle slot between its last fold and
            # the PE's final mask-matmul (W landed ~31.8us, DVE free ~32.0us)
            vector.wait_ge(dma_w, 16)
            vector.sem_clear(dma_w)
            vector.tensor_copy(w16[:], w_sb[:]).then_inc(v_w16, 1)
            vector.wait_ge(pe_sem, 1)
            vector.tensor_copy(s16[:], psum_s[:]).then_inc(v_sem, 1)
            vector.wait_ge(pe_sem, 2)
            vector.sem_clear(pe_sem)
            vector.tensor_copy(out_sb[:], psum_o[:]).then_inc(v_out, 1)

        @block.tensor
        def _(tensor):
            # s[d, b] += sum_p red_c[p, d] * mask[p, b], accumulated over chunks
            for c in range(NCHUNK):
                tensor.wait_ge(v_red, c + 1)
                mm = tensor.matmul(
                    psum_s[:],
                    xh[c][:, :DIN],
                    mask_sb[:],
                    start=(c == 0),
                    stop=(c == NCHUNK - 1),
                )
            tensor.sem_clear(v_red)
            mm.then_inc(pe_sem, 1)
            tensor.wait_ge(v_w16, 1)
            tensor.sem_clear(v_w16)
            tensor.wait_ge(v_sem, 1)
            tensor.sem_clear(v_sem)
            # out[b, jd] = sum_d s[d, b] * W[d, jd]
            tensor.matmul(
                psum_o[:], s16[:], w16[:], start=True, stop=True
            ).then_inc(pe_sem, 1)

    return nc


def _get_nc():
    if "nc" not in _cache:
        _cache["nc"] = _build_nc()
    return _cache["nc"]


def _in_maps(x, W):
    x = np.ascontiguousarray(x, dtype=np.float32)
    W = np.ascontiguousarray(W, dtype=np.float32)
    return [{"x": x[i * BSH : (i + 1) * BSH], "W": W} for i in range(N_CORES)]


def kernel(x, W, **profile_kwargs):
    nc = _get_nc()
    res = run_bass_kernel_spmd(nc, _in_maps(x, W), list(range(N_CORES)), **profile_kwargs)
    out = np.concatenate([r["out"] for r in res.results], axis=0)
    ret = out.reshape(B, 10, 16).astype(np.float32)
    if profile_kwargs:
        ret = (ret, res)
    return ret


# revision 22
# speedup vs baseline: 1.0033x; 1.0033x over previous
"""Trainium2 Bass kernel for nn_Capsule_16484084482446.

Reference math collapses: with cw = softmax(rw, axis=1),
  outputs[b,j,d] = sum_i sum_n cw[b,i,n] * u[b,j,n,d]
                 = sum_n u[b,j,n,d]           (since sum_i cw[b,i,n] == 1)
so the routing loop is a no-op and the final result is
  out = (sum_n x[b,n,:]) @ W   reshaped to (B, 10, 16).

Kernel strategy (data-parallel over batch, 4 batches per core x 8 cores):
  per core: x_shard (4, 4096, 128) viewed as 128 partitions x (128 rows x 128 d);
  partition p holds rows [128p, 128p+128), so batch b owns partitions [32b, 32b+32).

v2 final (baseline fp32 at 39.9us -> ~36.2us min; measured-trace findings):
  1. x chunks stream on the single sync HWDGE queue IN ORDER: the 16 DMA
     engines saturate (~340-376 GB/s run-to-run) either way, and a single
     queue makes chunk c complete at its stream-position time - splitting
     chunks across both HWDGE queues halved each queue's rate and DOUBLED
     every chunk's completion latency (v2b regression, +2.7us).
  2. DVE fold level 1 writes fp16 (fp32+fp32->fp16 add, measured 690ns
     per 512-elem level); levels 2+ run at the 16-bit 2-elem/cycle rate
     (415/283/215ns vs fp32 684/418/284). Fold work ~21us -> ~14us,
     under the ~22us stream window, so folds track the stream with no
     end-of-stream backlog (baseline lost ~3us here).
     gpsimd folding does NOT help: measured 2.5x slower than DVE and its
     SBUF port pressure slowed concurrent DVE ops up to 4x (v2b).
  3. All matmuls fp16 single-pass: mask-matmul LDWEIGHTS+MATMUL measured
     105+164ns vs fp32 dual-pass 625+700ns; final W-matmul 80+292ns.
     PSUM accumulates fp32.
  4. Chunk schedule minimizes max_c(land_c + sem_prop + suffix folds):
     smooth 16->8->4->2 taper; the last fold is one 284ns op, so the DVE
     finishes ~0.8us after the last byte lands.
  5. W is the LAST DMA on the sync queue (lands ~31.8us); the DVE casts
     it to fp16 in its idle slot between its last fold and the PE's
     final mask-matmul. W anywhere earlier delays the x stream (scalar-
     queue-first: +0.45us; gpsimd SWDGE cast-DMA at head: +4us). Beware:
     a first activation op on the Scalar engine loads the ACT table
     (~1.5us) - don't put the cast there.
  6. No wait on the out-DMA completion semaphore: the block-exit DRAIN
     (measured 160ns) flushes the queue, overlapping the exit barrier
     with the out-DMA flight (saves the ~1.2us land+sem wait).
  Post-fold chain (last byte -> out-DMA issued) is 3.3us; preamble 6.8us
  and exit accounting ~1.5us are runtime-fixed. Run-to-run HBM bandwidth
  varies +-5% (shared chip), so exec time spreads ~36.2-39.8us.
  Accuracy: fp16 folds of ~N(0,1) data with fp32 PSUM accumulation ->
  rel err ~6e-4 measured (gate 2e-2; inputs are seeded the same locally
  and in the harness, so a local pass is deterministic).

Raw Bass (no TileContext): Tile's tail drain needs more sync-wait slots than the
TRN2 CTRL encoding allows for this DMA-lane mix, and its end-of-kernel barriers
would dominate a ~36 us kernel. Every semaphore is cleared by its final consumer
right after its last wait, so the NEFF re-executes cleanly (profilers loop it).
"""

from contextlib import ExitStack

import numpy as np

import concourse.bass as bass
from concourse import mybir
from concourse.bass_utils import run_bass_kernel_spmd

N_CORES = 8
B, N, DIN = 32, 4096, 128
BSH = B // N_CORES          # 4 batches per core
DOUT = 160                  # 10 capsules * 16 dims
# rows-per-partition split: smooth 16->8->4->2 taper. Chosen to minimize
# max_c(land_c + sem_prop + sum of fold times from c to end): every chunk's
# fold finishes right as the next lands, and the last fold is one 284ns op,
# so the DVE is done 0.77us after the last byte (the schedule's floor).
CHUNKS = [8, 8, 16, 16, 16, 16, 8, 8, 8, 4, 4, 4, 4, 2, 2, 2, 2]
assert sum(CHUNKS) == BSH * N // 128
NCHUNK = len(CHUNKS)

F32 = mybir.dt.float32
F16 = mybir.dt.float16

_cache = {}


def _fold(eng, xc_c, xh_c, rows):
    """Halving fold of xc_c (fp32, rows*DIN) into xh_c[:, :DIN] (fp16).
    Level 1 casts fp32->fp16; later levels run at the 2x 16-bit rate."""
    s = rows // 2
    op = eng.tensor_add(
        xh_c[:, : s * DIN], xc_c[:, : s * DIN], xc_c[:, s * DIN :]
    )
    while s > 1:
        s //= 2
        op = eng.tensor_add(
            xh_c[:, : s * DIN],
            xh_c[:, : s * DIN],
            xh_c[:, s * DIN : 2 * s * DIN],
        )
    return op


def _build_nc(chunks=None, out_wait=False):
    global CHUNKS, NCHUNK
    if chunks is not None:
        CHUNKS = chunks
        NCHUNK = len(CHUNKS)
    assert sum(CHUNKS) == BSH * N // 128
    nc = bass.Bass()
    x = nc.dram_tensor("x", [BSH, N, DIN], F32, kind="ExternalInput")
    w = nc.dram_tensor("W", [DIN, DOUT], F32, kind="ExternalInput")
    out = nc.dram_tensor("out", [BSH, DOUT], F32, kind="ExternalOutput")

    # (128, 128, 128): partition p, row-in-partition n, feature d
    x3 = x[:].flatten_outer_dims().rearrange("(p n) d -> p n d", p=128)
    starts = np.cumsum([0] + CHUNKS).tolist()

    with ExitStack() as ctx:
        ec = ctx.enter_context
        xc = [ec(nc.sbuf_tensor(f"xc{c}", [128, CHUNKS[c] * DIN], F32))
              for c in range(NCHUNK)]
        xh = [ec(nc.sbuf_tensor(f"xh{c}", [128, (CHUNKS[c] // 2) * DIN], F16))
              for c in range(NCHUNK)]
        w_sb = ec(nc.sbuf_tensor("w_sb", [DIN, DOUT], F32))
        w16 = ec(nc.sbuf_tensor("w16", [DIN, DOUT], F16))
        mask_sb = ec(nc.sbuf_tensor("mask_sb", [128, BSH], F16))
        s16 = ec(nc.sbuf_tensor("s16", [DIN, BSH], F16))
        out_sb = ec(nc.sbuf_tensor("out_sb", [BSH, DOUT], F32))
        psum_s = ec(nc.psum_tensor("psum_s", [DIN, BSH], F32))
        psum_o = ec(nc.psum_tensor("psum_o", [BSH, DOUT], F32))

        dma_w = ec(nc.semaphore("dma_w"))
        dma_c = [ec(nc.semaphore(f"dma_c{c}")) for c in range(NCHUNK)]
        v_red = ec(nc.semaphore("v_red"))    # +1 per finished DVE fold
        v_w16 = ec(nc.semaphore("v_w16"))    # w16 landed (DMA sem, +16)
        pe_sem = ec(nc.semaphore("pe_sem"))
        v_sem = ec(nc.semaphore("v_sem"))    # s16 ready
        v_out = ec(nc.semaphore("v_out"))
        dma_out = ec(nc.semaphore("dma_out"))  # never waited (drain flushes)
        # Sem hygiene without an entry barrier: every semaphore is cleared by
        # its final consumer right after the consumer's last wait on it, so
        # every run (the profiler re-executes the NEFF) starts from zeros.
        # dma_out only ever grows; nothing waits on an absolute value.
        # no_gpsimd_drain: gpsimd issues no DMAs, so skip its expensive
        # dge_drain at block exit and use the sem-only barrier; Sync still
        # drains, which is what guarantees the out-DMA flushed
        block = ec(nc.Block(no_gpsimd_drain=True))

        @block.sync
        def _(sync):
            # W last: anywhere earlier delays the x stream (W-first on the
            # scalar HWDGE queue cost +0.45us; a gpsimd SWDGE cast-DMA at the
            # head cost +4us of stream delay). It lands ~31.8us, right when
            # the DVE goes idle to cast it.
            for c in range(NCHUNK):
                sync.dma_start(
                    xc[c][:], x3[:, starts[c] : starts[c + 1], :]
                ).then_inc(dma_c[c], 16)
            sync.dma_start(w_sb[:], w[:]).then_inc(dma_w, 16)
            sync.wait_ge(v_out, 1)
            sync.sem_clear(v_out)
            sync.dma_start(out[:], out_sb[:]).then_inc(dma_out, 16)
            if out_wait:
                sync.wait_ge(dma_out, 16)
                sync.sem_clear(dma_out)

        @block.vector
        def _(vector):
            # 0/1 batch mask, one 32-partition quadrant at a time (nonzero
            # partition bases only allow 32-partition windows)
            for q in range(4):
                for b in range(BSH):
                    vector.memset(
                        mask_sb[32 * q : 32 * (q + 1), b : b + 1],
                        1.0 if q == b else 0.0,
                    )
            for c in range(NCHUNK):
                vector.wait_ge(dma_c[c], 16)
                _fold(vector, xc[c], xh[c], CHUNKS[c]).then_inc(v_red, 1)
                # clear after the fold ops are queued - off the wait->fold path
                vector.sem_clear(dma_c[c])
            # w16 cast sits in the DVE's idle slot between its last fold and
            # the PE's final mask-matmul (W landed ~31.8us, DVE free ~32.0us)
            vector.wait_ge(dma_w, 16)
            vector.sem_clear(dma_w)
            vector.tensor_copy(w16[:], w_sb[:]).then_inc(v_w16, 1)
            vector.wait_ge(pe_sem, 1)
            vector.tensor_copy(s16[:], psum_s[:]).then_inc(v_sem, 1)
            vector.wait_ge(pe_sem, 2)
            vector.tensor_copy(out_sb[:], psum_o[:]).then_inc(v_out, 1)
            vector.sem_clear(pe_sem)

        @block.tensor
        def _(tensor):
            # s[d, b] += sum_p red_c[p, d] * mask[p, b], accumulated over chunks
            for c in range(NCHUNK):
                tensor.wait_ge(v_red, c + 1)
                mm = tensor.matmul(
                    psum_s[:],
                    xh[c][:, :DIN],
                    mask_sb[:],
                    start=(c == 0),
                    stop=(c == NCHUNK - 1),
                )
            mm.then_inc(pe_sem, 1)
            tensor.wait_ge(v_w16, 1)
            tensor.wait_ge(v_sem, 1)
            # out[b, jd] = sum_d s[d, b] * W[d, jd]
            tensor.matmul(
                psum_o[:], s16[:], w16[:], start=True, stop=True
            ).then_inc(pe_sem, 1)
            # sem hygiene moved off the critical path: clear after the final
            # matmul is dispatched, not between the stop-matmul and the LDW
            tensor.sem_clear(v_red)
            tensor.sem_clear(v_w16)
            tensor.sem_clear(v_sem)

    return nc


def _get_nc():
    if "nc" not in _cache:
        _cache["nc"] = _build_nc()
    return _cache["nc"]


def _in_maps(x, W):
    x = np.ascontiguousarray(x, dtype=np.float32)
    W = np.ascontiguousarray(W, dtype=np.float32)
    return [{"x": x[i * BSH : (i + 1) * BSH], "W": W} for i in range(N_CORES)]


def kernel(x, W, **profile_kwargs):
    nc = _get_nc()
    res = run_bass_kernel_spmd(nc, _in_maps(x, W), list(range(N_CORES)), **profile_kwargs)
    out = np.concatenate([r["out"] for r in res.results], axis=0)
    ret = out.reshape(B, 10, 16).astype(np.float32)
    if profile_kwargs:
        ret = (ret, res)
    return ret
